# revision 10
# baseline (speedup 1.0000x reference)
"""Trainium2 Bass kernel for nn_BistableHypergraphSynapse.

Reduced math:
    pre_spk    = mean_b x[b, :]
    pre_trace' = pre_trace*exp(-1/20) + pre_spk
    post_trace'= post_trace*exp(-1/20) + post_spk
    e_trace'   = 0.9*e_trace + pre_spk*(post_trace' + 0.1)
    eff        = s_e*w_hat
    g0[e,b]    = sum_{c: edge_c=e} x[b, node_c]
    gfw[e,b]   = eff[e]^2 * g0[e,b] / max(cnt[e],1)
    h[n,b]     = sum_{c: node_c=n} gfw[edge_c,b]
    out[b,n,o] = W[o]*h[n,b] + bias[o]

Device strategy (per the sharding hint: connections partitioned by index
range, tables sharded by range): each of the 8 cores owns a 12800-wide edge
range (phase B) and node range (phase C).  Connections are ELL-packed on the
host (grid row per edge/node, K=40 rank slots); the device fills the grid
with indirect-DMA gathers (one per partition: 4096 tokens land contiguously
in that partition), reduces the K axis with the vector engine, and exchanges
gfw chunks with a single AllGather.  No scatter hardware is used.
"""
import numpy as np

P = 128
CORES = 8
N = 100000
B = 4
OC = 16
RNG = 12800              # per-core index range (8*12800 = 102400 padded)
GPAD = RNG * CORES
K = 41                   # ELL rank slots per row (40 usable; 1 may be poisoned)
EPP = 100                # edge rows per partition (100*128 = 12800)
C = 1664                 # tokens per gather instruction (validated shape)
W = C // P               # 13 offset columns consumed per instruction
ROUNDS = 3               # gather instructions per partition (3*1664 >= 4100)
SPP = ROUNDS * C         # slots per partition (4992)
OCOLS = W * (P * ROUNDS - 1) + C   # offsets tile columns
# Every 104th token within a gather instruction (the per-SDMA-engine stream
# head, 26624B/16 = 1664B = 104 tokens) lands corrupted; those grid slots are
# never used and are zeroed on-device before the reduce.
_PK = np.full(100, 99, np.int64)
for _r in range(ROUNDS):
    for _k2 in range(16):
        if _r == 0 and _k2 == 0:
            continue
        _s = _r * C + _k2 * 104
        _jj, _kk = divmod(_s, 41)
        if _jj < 100 and _kk < 41:
            _PK[_jj] = _kk
ZROW = 102000            # all-zero row in both gather tables (pad target)
DECAY = float(np.exp(np.float32(-1.0 / 20.0)))

_CACHE = {}


def _build_nc():
    import concourse.bacc as bacc
    import concourse.mybir as mybir
    import concourse.tile as tile
    from concourse.bass import IndirectOffsetOnAxis

    f32 = mybir.dt.float32
    i32 = mybir.dt.int32
    nc = bacc.Bacc("TRN2", target_bir_lowering=False, debug=False, num_devices=CORES)

    xtp = nc.dram_tensor("xtp", [GPAD, 4], f32, kind="ExternalInput")
    xs_in = nc.dram_tensor("xs_in", [RNG, 4], f32, kind="ExternalInput")
    vs_in = nc.dram_tensor("vs_in", [7, RNG], f32, kind="ExternalInput")
    obo = nc.dram_tensor("obo", [P, OCOLS], i32, kind="ExternalInput")
    oco = nc.dram_tensor("oco", [P, OCOLS], i32, kind="ExternalInput")
    wb_in = nc.dram_tensor("wb_in", [P, 2, 100 * OC], f32, kind="ExternalInput")

    out_o = nc.dram_tensor("out_o", [B, RNG, OC], f32, kind="ExternalOutput")
    tr_o = nc.dram_tensor("tr_o", [3, RNG], f32, kind="ExternalOutput")

    agin = nc.dram_tensor("agin", [RNG, 4], f32)
    agout = nc.dram_tensor("agout", [GPAD, 4], f32, addr_space="Shared")

    with tile.TileContext(nc) as tc:
        with tc.tile_pool(name="pool", bufs=1) as pool:
            xs = pool.tile([P, 100, 4], f32, tag="xs")
            vs = pool.tile([P, 7, 100], f32, tag="vs")
            trt = pool.tile([P, 3, 100], f32, tag="trt")
            off_b = pool.tile([P, OCOLS], i32, tag="off")
            grid = pool.tile([P, SPP, 4], f32, tag="grid")
            red = pool.tile([P, 100, 4], f32, tag="red")
            sc0 = pool.tile([P, 100], f32, tag="sc0")
            sc1 = pool.tile([P, 100], f32, tag="sc1")
            sc2 = pool.tile([P, 100], f32, tag="sc2")
            eff2 = pool.tile([P, 100], f32, tag="eff2")
            agt = pool.tile([P, 100, 4], f32, tag="agt")
            wbt = pool.tile([P, 2, 100 * OC], f32, tag="wbt")

            # ---- loads ----
            nc.sync.dma_start(xs[:], xs_in[:].rearrange("(p c) e -> p c e", p=P))
            nc.sync.dma_start(vs[:], vs_in[:].rearrange("v (p c) -> p v c", p=P))
            nc.sync.dma_start(off_b[:], obo[:])
            nc.sync.dma_start(wbt[:], wb_in[:])

            # ---- phase 0: trace updates + eff^2 ----
            nc.vector.tensor_add(sc0[:], xs[:, :, 0], xs[:, :, 1])
            nc.vector.tensor_add(sc1[:], xs[:, :, 2], xs[:, :, 3])
            nc.vector.tensor_add(sc0[:], sc0[:], sc1[:])
            nc.vector.tensor_scalar_mul(sc0[:], sc0[:], 0.25)       # pre_spk
            nc.vector.tensor_scalar_mul(sc1[:], vs[:, 4, :], DECAY)
            nc.vector.tensor_add(trt[:, 1, :], sc1[:], sc0[:])      # pre_trace'
            nc.vector.tensor_scalar_mul(sc1[:], vs[:, 5, :], DECAY)
            nc.vector.tensor_add(trt[:, 2, :], sc1[:], vs[:, 0, :])  # post_trace'
            nc.vector.tensor_scalar_add(sc1[:], trt[:, 2, :], 0.1)
            nc.vector.tensor_mul(sc1[:], sc1[:], sc0[:])
            nc.vector.tensor_scalar_mul(sc2[:], vs[:, 3, :], 0.9)
            nc.vector.tensor_add(trt[:, 0, :], sc2[:], sc1[:])      # e_trace'
            nc.sync.dma_start(tr_o[:].rearrange("v (p c) -> p v c", p=P), trt[:])
            nc.vector.tensor_mul(eff2[:], vs[:, 1, :], vs[:, 2, :])
            nc.vector.tensor_mul(eff2[:], eff2[:], eff2[:])

            # ---- phase B: ELL gather of x by node, reduce to g0 ----
            for k in range(P):
                for r in range(ROUNDS):
                    m = k * ROUNDS + r
                    nc.gpsimd.indirect_dma_start(
                        out=grid[k : k + 1, r * C : (r + 1) * C, :],
                        out_offset=None,
                        in_=xtp[:],
                        in_offset=IndirectOffsetOnAxis(
                            ap=off_b[:, m * W : m * W + C], axis=0
                        ),
                    )
            pz = grid[:].rearrange("p (r k2 u) v -> p r k2 u v", k2=16, u=104)
            nc.vector.memset(pz[:, :, 1:16, 0, :], 0.0)
            nc.vector.memset(pz[:, 1:ROUNDS, 0:1, 0, :], 0.0)
            g4 = grid[:, 0 : EPP * K, :].rearrange("p (e k) v -> p e k v", k=K)
            nc.vector.tensor_copy(red[:], g4[:, 0:100, 0, :])
            for kk in range(1, K):
                nc.vector.tensor_add(red[:], red[:], g4[:, 0:100, kk, :])

            # ---- gfw = eff^2 * g0 / max(cnt,1); allgather ----
            nc.vector.tensor_scalar_max(sc0[:], vs[:, 6, :], 1.0)   # counts
            nc.vector.reciprocal(sc1[:], sc0[:])
            nc.vector.tensor_mul(sc1[:], sc1[:], eff2[:])
            for b in range(B):
                nc.vector.tensor_mul(agt[:, :, b], red[:, :, b], sc1[:])
            nc.sync.dma_start(agin[:].rearrange("(p c) e -> p c e", p=P), agt[:])
            nc.gpsimd.collective_compute(
                "AllGather",
                mybir.AluOpType.bypass,
                replica_groups=[list(range(CORES))],
                ins=[agin[:]],
                outs=[agout[:]],
            )

            # ---- phase C: ELL gather of gfw by edge, reduce to h ----
            off_c = pool.tile([P, OCOLS], i32, tag="off")
            grid2 = pool.tile([P, SPP, 4], f32, tag="grid")
            hred = pool.tile([P, 100, 4], f32, tag="red")
            nc.sync.dma_start(off_c[:], oco[:])
            for k in range(P):
                for r in range(ROUNDS):
                    m = k * ROUNDS + r
                    nc.gpsimd.indirect_dma_start(
                        out=grid2[k : k + 1, r * C : (r + 1) * C, :],
                        out_offset=None,
                        in_=agout[:],
                        in_offset=IndirectOffsetOnAxis(
                            ap=off_c[:, m * W : m * W + C], axis=0
                        ),
                    )
            pz2 = grid2[:].rearrange("p (r k2 u) v -> p r k2 u v", k2=16, u=104)
            nc.vector.memset(pz2[:, :, 1:16, 0, :], 0.0)
            nc.vector.memset(pz2[:, 1:ROUNDS, 0:1, 0, :], 0.0)
            h4 = grid2[:, 0 : EPP * K, :].rearrange("p (e k) v -> p e k v", k=K)
            nc.vector.tensor_copy(hred[:], h4[:, 0:100, 0, :])
            for kk in range(1, K):
                nc.vector.tensor_add(hred[:], hred[:], h4[:, 0:100, kk, :])

            # ---- output: out[b, n, o] = W[o]*h[n, b] + bias[o] ----
            for b in range(B):
                ob = pool.tile([P, 100, OC], f32, tag=f"ob{b % 2}")
                hb = hred[:, :, b : b + 1].to_broadcast([P, 100, OC])
                nc.vector.tensor_mul(
                    ob[:], hb, wbt[:, 0, :].rearrange("p (c o) -> p c o", o=OC)
                )
                nc.vector.tensor_add(
                    ob[:], ob[:], wbt[:, 1, :].rearrange("p (c o) -> p c o", o=OC)
                )
                nc.sync.dma_start(out_o[b].rearrange("(p c) o -> p c o", p=P), ob[:])
    nc.compile()
    return nc


def _get_nc():
    if "nc" not in _CACHE:
        _CACHE["nc"] = _build_nc()
    return _CACHE["nc"]


def _ell_offsets(local_idx, values):
    """Offsets array (P, W*P + C) for one core+phase.

    local_idx: local row id (0..12799) per conn
    values: gather offset per conn (node_idx or global edge row)
    """
    arr = np.full((P, OCOLS), ZROW, np.int32)
    order = np.argsort(local_idx, kind="stable")
    le = local_idx[order]
    v = values[order]
    # rank within each row
    cnt = np.bincount(le, minlength=RNG)
    excl = np.concatenate([[0], np.cumsum(cnt)[:-1]])
    rank = np.arange(len(le)) - excl[le]
    p = le // EPP
    jj = le % EPP
    kk = rank + (rank >= _PK[jj])     # skip the poisoned slot of this row
    s = jj * K + kk                   # slot within partition (0..4099)
    m = p * ROUNDS + s // C           # instruction id
    t = s % C                         # token within instruction
    arr[t % P, m * W + t // P] = v
    return arr


def kernel(x_in, hyperedge_index, post_spk, w_hat, s_e, e_trace, pre_trace,
           post_trace, weight_node, bias):
    from concourse.bass_utils import run_bass_kernel_spmd

    x_in = np.ascontiguousarray(np.asarray(x_in, dtype=np.float32))
    hei = np.asarray(hyperedge_index)
    node_idx = np.asarray(hei[0], dtype=np.int64)
    edge_idx = np.asarray(hei[1], dtype=np.int64)
    post_spk = np.asarray(post_spk, dtype=np.float32)
    w_hat = np.asarray(w_hat, dtype=np.float32)
    s_e = np.asarray(s_e, dtype=np.float32)
    e_trace = np.asarray(e_trace, dtype=np.float32)
    pre_trace = np.asarray(pre_trace, dtype=np.float32)
    post_trace = np.asarray(post_trace, dtype=np.float32)
    W_ = np.asarray(weight_node, dtype=np.float32)[0]
    bias_v = np.asarray(bias, dtype=np.float32)

    xtp = np.zeros((GPAD, 4), np.float32)
    xtp[:N] = x_in.T
    ecnt = np.bincount(edge_idx, minlength=GPAD).astype(np.float32)
    assert ecnt.max() <= K - 1, ecnt.max()
    assert np.bincount(node_idx, minlength=1).max() <= K - 1
    vecs = np.zeros((7, GPAD), np.float32)
    for i, v in enumerate([post_spk, s_e, w_hat, e_trace, pre_trace, post_trace]):
        vecs[i, : len(v)] = v
    vecs[6] = ecnt
    wb = np.zeros((2, 100 * OC), np.float32)
    wb[0] = np.tile(W_, 100)
    wb[1] = np.tile(bias_v, 100)
    wb_rep = np.broadcast_to(wb[None], (P, 2, 100 * OC)).copy()

    eb = edge_idx // RNG
    nb = node_idx // RNG
    in_maps = []
    for j in range(CORES):
        mb_ = eb == j
        mc_ = nb == j
        le_b = (edge_idx[mb_] - j * RNG).astype(np.int64)
        le_c = (node_idx[mc_] - j * RNG).astype(np.int64)
        in_maps.append({
            "xtp": xtp,
            "xs_in": np.ascontiguousarray(xtp[j * RNG : (j + 1) * RNG]),
            "vs_in": np.ascontiguousarray(vecs[:, j * RNG : (j + 1) * RNG]),
            "obo": _ell_offsets(le_b, node_idx[mb_].astype(np.int32)),
            "oco": _ell_offsets(le_c, edge_idx[mc_].astype(np.int32)),
            "wb_in": wb_rep,
        })

    nc = _get_nc()
    res = run_bass_kernel_spmd(nc, in_maps, core_ids=list(range(CORES)))
    _CACHE["last_res"] = res

    out = np.concatenate([r["out_o"] for r in res.results], axis=1)[:, :N, :]
    trs = np.concatenate([r["tr_o"] for r in res.results], axis=1)[:, :N]
    return out, trs[0], trs[1], trs[2]
